# revision 17
# baseline (speedup 1.0000x reference)
"""MeshGraphNet on 8 Trainium2 NeuronCores (Bass/Tile, SPMD).

Strategy (see spec sharding_hint): edges partitioned across cores by
*receiver* node; node space padded 10000 -> 10240 and bin-packed into
8 cores x 10 windows x 128 nodes so each window owns <= 768 incident edges
(6 edge-tiles of 128). All matmul data is fp16 (PE: 1 cycle/row, 8x finer
mantissa than bf16) and flows feature-major: MLP weights are the
stationary lhsT and the activations are wide moving rhs. Receiver gather /
scatter-add are one-hot matmuls fused into the PSUM accumulation of the
consuming/producing MLP stage. The sender gather runs on the
W1b-transformed node embedding Z = hn @ pe_W1[128:256] + b1, AllGather'ed
to DRAM once per layer (fp16) and row-gathered by sender index via SWDGE
dma_gather in transpose mode.

Engine diet vs the v1 kernel: each edge window is ONE G=6 MLP group;
all rank-1 biases (b3, zb1, beta/g) are folded into PSUM accumulations
as K=1 matmuls; LayerNorm stats come from tensor_reduce (sum) + squared
tensor_tensor + tensor_reduce instead of per-tile bn_stats/bn_aggr; the
normalize is y*A + B with per-edge A/B and the LN gain g is folded into
the consumers (aggT evacuation scale / residual STT scale), so the DVE
does 5 full passes per window instead of ~10. Residual state is fp16.
"""

import os
import sys

import numpy as np

for _p in ("/opt/trn_rl_repo", "/root/.axon_site/_ro/trn_rl_repo"):
    if os.path.isdir(_p) and _p not in sys.path:
        sys.path.insert(0, _p)

import concourse.bass as bass
import concourse.bacc as bacc
import concourse.mybir as mybir
import concourse.tile as tile
from concourse.bass_utils import run_bass_kernel_spmd

F32 = mybir.dt.float32
BF16 = mybir.dt.float16  # 16-bit compute dtype (fp16: 1 cyc/row on PE)
I16 = mybir.dt.int16
ALU = mybir.AluOpType
ACT_F = mybir.ActivationFunctionType

N, E, D = 10000, 60000, 128
NF, EF, NL = 12, 3, 15
CORES = 8
WPC = 10                       # windows per core
NPC = WPC * 128                # 1280 nodes per core
NPAD = CORES * NPC             # 10240
TPW = 6                        # edge tiles per window
CAP = TPW * 128                # 768 edges per window max
T = WPC * TPW                  # 60 edge tiles per core
P = T * 128                    # 7680 edge slots per core
EPS = 1e-5

NGRP = [(0, 4), (4, 3), (7, 3)]             # node window groups
EGRP = [(t0, 6) for t0 in range(0, T, 6)]   # edge-encoder tile groups

# Rw row-constant layout (fp16 [1, 1792] per layer)
RW_B3E = 0          # b3e tiled 6x  [768]
RW_B3N = 768        # b3n tiled 4x  [512]
RW_ZB1 = 1280       # adjusted pe_b1 tiled 4x [512]
RW_LEN = 1792

# encRw row-constant layout (fp16 [1, 1552])
ERW_B3N = 0         # enc_n_b3 tiled 6x [768]
ERW_B3E = 768       # enc_e_b3 tiled 6x [768]
ERW_B3D = 1536      # dec_b3 tiled 4x [12] (+pad)
ERW_LEN = 1552


# ----------------------------------------------------------------------------
# Host-side graph packing
# ----------------------------------------------------------------------------

def pack_graph(edge_index):
    send0 = np.asarray(edge_index[0], np.int64)
    recv0 = np.asarray(edge_index[1], np.int64)
    deg = np.bincount(recv0, minlength=N)

    order = np.argsort(-deg, kind="stable")
    nwin = CORES * WPC
    win_fill = np.zeros(nwin, dtype=np.int64)    # node count per window
    win_load = np.zeros(nwin, dtype=np.int64)    # edge count per window
    perm_pos = np.full(N, -1, dtype=np.int64)
    for nid in order:
        d = deg[nid]
        cand = np.nonzero((win_fill < 128) & (win_load + d <= CAP))[0]
        assert len(cand), "window packing failed (need TPW bump)"
        w = cand[np.argmin(win_load[cand])]
        perm_pos[nid] = w * 128 + win_fill[w]
        win_fill[w] += 1
        win_load[w] += d
    assert (perm_pos >= 0).all()

    send_new = perm_pos[send0]
    recv_new = perm_pos[recv0]

    ewin = recv_new // 128
    edge_slots = np.full((CORES, P), -1, dtype=np.int64)
    for c in range(CORES):
        for wl in range(WPC):
            w = c * WPC + wl
            eids = np.nonzero(ewin == w)[0]
            base = wl * CAP
            edge_slots[c, base : base + len(eids)] = eids
    return perm_pos, send_new, recv_new, edge_slots


def _col(v):
    """[K] -> [K, 1] fp32 column (per-partition bias/scale)."""
    return np.asarray(v, np.float32).reshape(-1, 1).copy()


def build_inputs(inp, perm_pos, send_new, recv_new, edge_slots):
    """Build in_maps (one dict per core) for the device program."""
    g32 = lambda k: np.ascontiguousarray(np.asarray(inp[k], np.float32))
    bf = lambda a: np.asarray(a, np.float32).astype(np.float16)

    nf_pad = np.zeros((NPAD, NF), np.float32)
    nf_pad[perm_pos] = g32("node_features")
    ef = g32("edge_features")

    peW1, peW2, peW3 = g32("pe_W1"), g32("pe_W2"), g32("pe_W3")
    pnW1, pnW2, pnW3 = g32("pn_W1"), g32("pn_W2"), g32("pn_W3")

    # per-layer stationary/moving weights, fp16, [NL, 128, 9*128]
    Wp = np.stack([
        np.concatenate([
            peW1[l, 0:128], peW1[l, 256:384], peW1[l, 128:256],
            peW2[l], peW3[l],
            pnW1[l, 0:128], pnW1[l, 128:256], pnW2[l], pnW3[l],
        ], axis=1)
        for l in range(NL)
    ])  # order: W1a W1c W1b W2e W3e W1n0 W1n1 W2n W3n

    def tl(v, n):
        return np.tile(np.asarray(v, np.float32).reshape(1, -1), (1, n))

    # Device state is beta-free: he'_l = he_l - C_l, hn'_l = hn_l - D_l
    # with C_l = sum_{k<l} pe_beta_k, D_l = sum_{k<l} pn_beta_k. The
    # constant parts are folded into downstream bias columns here.
    betae = np.asarray(inp["pe_beta"], np.float64)
    betan = np.asarray(inp["pn_beta"], np.float64)
    Cl = np.concatenate([np.zeros((1, D)), np.cumsum(betae, 0)])[:NL]
    Dl = np.concatenate([np.zeros((1, D)), np.cumsum(betan, 0)])[:NL]
    D_NL = betan.sum(0)

    # per-layer fp16 rows [1, RW_LEN]
    Rw = np.stack([
        np.concatenate([
            tl(inp["pe_b3"][l], 6), tl(inp["pn_b3"][l], 4),
            tl(np.asarray(inp["pe_b1"][l], np.float64)
               + Dl[l] @ np.asarray(peW1[l, 128:256], np.float64), 4),
        ], axis=1)
        for l in range(NL)
    ])  # [NL, 1, RW_LEN]

    # per-layer fp32 columns [128, 6]: b2e, b1n', b2n, ge, gn, ey1b
    Cc = np.stack([
        np.concatenate([
            _col(inp["pe_b2"][l]),
            _col(np.asarray(inp["pn_b1"][l], np.float64)
                 + Dl[l] @ np.asarray(pnW1[l, 0:128], np.float64)),
            _col(inp["pn_b2"][l]),
            _col(inp["pe_g"][l]), _col(inp["pn_g"][l]),
            _col(Cl[l] @ np.asarray(peW1[l, 0:128], np.float64)
                 + Dl[l] @ np.asarray(peW1[l, 256:384], np.float64)),
        ], axis=1)
        for l in range(NL)
    ])
    # per-layer single-partition row [1, 128]: betaW1 (deg compensation)
    betaW1 = np.stack([
        np.asarray(inp["pe_beta"][l], np.float64) @ np.asarray(pnW1[l, 128:256], np.float64)
        for l in range(NL)
    ]).astype(np.float32)
    R1 = betaW1.reshape(NL, 1, 128)

    encRw = np.concatenate([
        tl(inp["enc_n_b3"], 6), tl(inp["enc_e_b3"], 6),
        tl(inp["dec_b3"], 4), np.zeros((1, 4), np.float32),
    ], axis=1)  # [1, ERW_LEN]

    shared = {
        "Wp": bf(Wp), "Rw": bf(Rw), "Cc": Cc, "R1": bf(R1),
        # encoders / decoder (fp16, like the layer loop)
        "encnW1": bf(g32("enc_n_W1")),
        "enceW1": bf(g32("enc_e_W1")),
        "encW": bf(np.concatenate([
            g32("enc_n_W2"), g32("enc_n_W3"),
            g32("enc_e_W2"), g32("enc_e_W3"),
            g32("dec_W1"), g32("dec_W2"),
        ], axis=1)),  # [128, 6*128]
        "decW3": bf(g32("dec_W3")),
        # [128, 10] fp32: encn_b1 b2 g beta | ence_b1 b2 g beta | dec_b1' b2
        "encC": np.concatenate([
            _col(inp["enc_n_b1"]), _col(inp["enc_n_b2"]),
            _col(inp["enc_n_g"]), _col(inp["enc_n_beta"]),
            _col(inp["enc_e_b1"]), _col(inp["enc_e_b2"]),
            _col(inp["enc_e_g"]), _col(inp["enc_e_beta"]),
            _col(np.asarray(inp["dec_b1"], np.float64)
                 + D_NL @ np.asarray(g32("dec_W1"), np.float64)),
            _col(inp["dec_b2"]),
        ], axis=1),
        "encRw": bf(encRw),
        "ones": bf(np.ones((1, CAP), np.float32)),
        "ident": bf(np.eye(128, dtype=np.float32)),
    }

    in_maps = []
    for c in range(CORES):
        sl = edge_slots[c]
        v = sl >= 0
        send_c = np.zeros(P, np.int64)
        send_c[v] = send_new[sl[v]]
        recv_c = np.zeros(P, np.int64)
        recv_c[v] = recv_new[sl[v]]

        efT = np.zeros((EF, P), np.float32)
        efT[:, v] = ef[sl[v]].T

        slots = np.arange(P)
        t, p = slots // 128, slots % 128
        j = recv_c - (c * NPC + (t // TPW) * 128)
        O = np.zeros((128, P), np.float32)
        OT = np.zeros((128, P), np.float32)
        O[p[v], t[v] * 128 + j[v]] = 1.0
        OT[j[v], t[v] * 128 + p[v]] = 1.0

        wrapped = send_c.astype(np.int16).reshape(P // 16, 16).T  # [16, P//16]
        sidx = np.tile(wrapped, (8, 1))                           # [128, P//16]

        # per-local-node incident edge count (receiver degree), [1, NPC]
        deg_c = np.zeros(NPC, np.float32)
        np.add.at(deg_c, recv_c[v] - c * NPC, 1.0)

        m = dict(shared)
        m.update({
            "nfT": bf(np.ascontiguousarray(
                nf_pad[c * NPC:(c + 1) * NPC].T).reshape(NF, WPC, 128)),
            "efT": bf(efT.reshape(EF, T, 128)),
            "O": bf(O.reshape(128, T, 128)),
            "OT": bf(OT.reshape(128, T, 128)),
            "sidx": sidx,
            "deg": bf(deg_c.reshape(1, NPC)),
        })
        in_maps.append(m)
    return in_maps


# ----------------------------------------------------------------------------
# Device program
# ----------------------------------------------------------------------------

def build_program(n_layers=NL, sim1=False):
    nc = bacc.Bacc("TRN2", target_bir_lowering=False, debug=False,
                   num_devices=1 if sim1 else CORES)

    dram = {}

    def din(name, shape, dt=BF16):
        dram[name] = nc.dram_tensor(name, list(shape), dt, kind="ExternalInput")
        return dram[name]

    din("nfT", [NF, WPC, 128])
    din("efT", [EF, T, 128])
    din("O", [128, T, 128])
    din("OT", [128, T, 128])
    din("sidx", [128, P // 16], I16)
    din("deg", [1, NPC])
    din("Wp", [NL, 128, 9 * 128])
    din("Rw", [NL, 1, RW_LEN])
    din("Cc", [NL, 128, 6], F32)
    din("R1", [NL, 1, 128])
    din("encnW1", [NF, 128])
    din("enceW1", [EF, 128])
    din("encW", [128, 6 * 128])
    din("decW3", [128, 3])
    din("encC", [128, 10], F32)
    din("encRw", [1, ERW_LEN])
    din("ones", [1, CAP])
    din("ident", [128, 128])
    out_d = nc.dram_tensor("out", [NPC, 3], F32, kind="ExternalOutput")

    with tile.TileContext(nc) as tc:
        _build_tile_program(nc, tc, dram, out_d, n_layers, sim1)
    nc.compile()
    return nc


def _build_tile_program(nc, tc, dram, out_d, n_layers, sim1=False):
    from contextlib import ExitStack

    st = ExitStack()
    const = st.enter_context(tc.tile_pool(name="const", bufs=1))
    wpool = st.enter_context(tc.tile_pool(name="wpool", bufs=2))
    work = st.enter_context(tc.tile_pool(name="work", bufs=4))
    xpool = st.enter_context(tc.tile_pool(name="xpool", bufs=6))
    ps_mlp = st.enter_context(tc.tile_pool(name="ps_mlp", bufs=2, space="PSUM"))
    ps_tp = st.enter_context(tc.tile_pool(name="ps_tp", bufs=2, space="PSUM"))
    ps_sm = st.enter_context(tc.tile_pool(name="ps_sm", bufs=2, space="PSUM"))
    dpool = st.enter_context(tc.tile_pool(name="dram", bufs=2, space="DRAM"))

    NO_CC = bool(int(os.environ.get("K_NO_CC", "0")))
    NO_GATHER = bool(int(os.environ.get("K_NO_GATHER", "0")))
    zspace = "Local" if (sim1 or NO_CC) else "Shared"

    def mm(out, lhsT, rhs, start=True, stop=True):
        nc.tensor.matmul(out, lhsT, rhs, start=start, stop=stop)

    def cs(ap2, c):   # chunk slice: [..., K, 128*nc] -> cols of chunk c
        return ap2[:, c * 128:(c + 1) * 128]

    # ---- resident SBUF state ----
    ident = const.tile([128, 128], BF16)
    nc.sync.dma_start(ident[:], dram["ident"][:])
    ones = const.tile([1, CAP], BF16)
    nc.sync.dma_start(ones[:], dram["ones"][:])
    deg = const.tile([1, NPC], BF16)
    nc.sync.dma_start(deg[:], dram["deg"][:])
    O_sb = const.tile([128, T, 128], BF16)
    nc.sync.dma_start(O_sb[:], dram["O"][:])
    OT_sb = const.tile([128, T, 128], BF16)
    nc.sync.dma_start(OT_sb[:], dram["OT"][:])
    sidx = const.tile([128, P // 16], I16)
    nc.sync.dma_start(sidx[:], dram["sidx"][:])
    eps_col = const.tile([128, 1], F32)
    nc.vector.memset(eps_col[:], EPS)
    zeros_col = const.tile([128, 1], BF16)
    nc.vector.memset(zeros_col[:], 0.0)
    he_fm = const.tile([128, T, 128], BF16)    # edge state, feature-major
    hnT = const.tile([128, WPC, 128], BF16)    # node state, feature-major
    aggT = const.tile([128, WPC, 128], BF16)   # scatter result, feature-major

    # ---- encoder/decoder weights (fp16) ----
    encnW1 = const.tile([NF, 128], BF16)
    nc.sync.dma_start(encnW1[:], dram["encnW1"][:])
    enceW1 = const.tile([EF, 128], BF16)
    nc.sync.dma_start(enceW1[:], dram["enceW1"][:])
    encW = const.tile([128, 6 * 128], BF16)
    nc.sync.dma_start(encW[:], dram["encW"][:])
    decW3 = const.tile([128, 3], BF16)
    nc.sync.dma_start(decW3[:], dram["decW3"][:])
    encC = const.tile([128, 10], F32)
    nc.sync.dma_start(encC[:], dram["encC"][:])
    encRw = const.tile([1, ERW_LEN], BF16)
    nc.sync.dma_start(encRw[:], dram["encRw"][:])
    nfT = const.tile([NF, WPC, 128], BF16)
    nc.sync.dma_start(nfT[:], dram["nfT"][:])
    efT = const.tile([EF, T, 128], BF16)
    nc.sync.dma_start(efT[:], dram["efT"][:])
    ones1 = ones[:, 0:128]

    def weight_tiles(l):
        Wp = wpool.tile([128, 9 * 128], BF16, tag="Wp")
        nc.sync.dma_start(Wp[:], dram["Wp"][l])
        Rw = wpool.tile([1, RW_LEN], BF16, tag="Rw")
        nc.sync.dma_start(Rw[:], dram["Rw"][l])
        Cc = wpool.tile([128, 6], F32, tag="Cc")
        nc.sync.dma_start(Cc[:], dram["Cc"][l])
        R1 = wpool.tile([1, 128], BF16, tag="R1")
        nc.sync.dma_start(R1[:], dram["R1"][l])
        return {"Wp": Wp, "Rw": Rw, "Cc": Cc, "R1": R1}

    def mlp3(parts, b1_col, W2, b2_col, W3, b3_row, G):
        """3-stage MLP on G tiles; stage-1 inputs are feature-major
        (lhsT, slicer) pairs accumulated into one PSUM, where
        slicer(a, b) gives the moving operand for tile range [a, b).
        Matmuls are chunked to <=512 fp32 out cols (one PSUM bank).
        Output ps3 is edge/node-major [128, G, 128] fp32 PSUM with b3
        already added (b3_row is [1, G*128], a K=1 rank-1 term)."""
        chunks = [(0, min(G, 4))] + ([(4, G)] if G > 4 else [])
        ps1 = ps_mlp.tile([128, G, 128], F32, tag="mlp")
        nmm = len(parts)
        for a, b in chunks:
            for i, (lt, sl) in enumerate(parts):
                mm(ps1[:, a:b, :], lt, sl(a, b),
                   start=(i == 0), stop=(i == nmm - 1))
        y1 = work.tile([128, G, 128], BF16, tag="y1")
        b1s = 0.0 if b1_col is None else b1_col
        nc.vector.scalar_tensor_tensor(
            y1[:], ps1[:], b1s, zeros_col[:, None, :].to_broadcast([128, G, 128]),
            ALU.add, ALU.max)
        ps2 = ps_mlp.tile([128, G, 128], F32, tag="mlp")
        for a, b in chunks:
            mm(ps2[:, a:b, :], W2, y1[:, a:b, :])
        y2 = work.tile([128, G, 128], BF16, tag="y2")
        nc.scalar.activation(y2[:], ps2[:], ACT_F.Relu, bias=b2_col)
        ps3 = ps_mlp.tile([128, G, 128], F32, tag="mlp")
        for i in range(G):
            mm(ps3[:, i, :], ones1, b3_row[:, i * 128:(i + 1) * 128],
               start=True, stop=False)
            mm(ps3[:, i, :], y2[:, i, :], W3, start=False, stop=True)
        return ps3

    def ln_ab(ps3, G):
        """LN coefficients per row of [128, G, 128]: A = rstd,
        B = -mean*rstd, both [128, G, 1] fp32."""
        s1 = work.tile([128, G, 1], F32, tag="s1")
        nc.vector.tensor_reduce(s1[:], ps3[:], mybir.AxisListType.X, ALU.add)
        sq = work.tile([128, G, 128], BF16, tag="sq")
        nc.scalar.activation(sq[:], ps3[:], ACT_F.Square)
        s2 = work.tile([128, G, 1], F32, tag="s2")
        nc.vector.tensor_reduce(s2[:], sq[:], mybir.AxisListType.X, ALU.add)
        # var*128^2 = 128*sumsq - sum^2
        t1 = work.tile([128, G, 1], F32, tag="t1")
        nc.vector.tensor_tensor(t1[:], s1[:], s1[:], ALU.mult)
        vs = work.tile([128, G, 1], F32, tag="vs")
        nc.vector.scalar_tensor_tensor(vs[:], s2[:], 128.0, t1[:],
                                       ALU.mult, ALU.subtract)
        sd = work.tile([128, G, 1], F32, tag="sd")
        nc.scalar.activation(sd[:], vs[:], ACT_F.Sqrt, bias=eps_col[:],
                             scale=1.0 / 16384.0)
        A = work.tile([128, G, 1], F32, tag="A")
        nc.vector.reciprocal(A[:], sd[:])
        B = work.tile([128, G, 1], F32, tag="B")
        nc.vector.scalar_tensor_tensor(B[:], s1[:], -1.0 / 128.0, A[:],
                                       ALU.mult, ALU.mult)
        return A, B

    def norm(ps3, A, B, G):
        """(ps3 - mean) * rstd -> fp16 [128, G, 128] (no LN gain g)."""
        xng = work.tile([128, G, 128], BF16, tag="xng")
        nc.vector.tensor_tensor(
            xng[:], ps3[:], A[:].to_broadcast([128, G, 128]), ALU.mult)
        nc.vector.tensor_tensor(
            xng[:], xng[:], B[:].to_broadcast([128, G, 128]), ALU.add)
        return xng

    def transpose_g(xng, G):
        """psT = xng^T per tile, [128, G, 128] fp16 PSUM (feature-major)."""
        psT = ps_tp.tile([128, G, 128], BF16, tag="tp")
        for i in range(G):
            nc.tensor.transpose(psT[:, i, :], xng[:, i, :], ident[:])
        return psT

    def z_alloc():
        zin = dpool.tile([NPC, 128], BF16, tag="zin")
        zout = dpool.tile([NPAD, 128], BF16, tag="zout", addr_space=zspace)
        return zin, zout

    def z_group(w0, G, zin, wt):
        """Z = hn @ W1b + b1e for one node group -> zin rows."""
        W1b = cs(wt["Wp"], 2)
        zb1 = wt["Rw"][:, RW_ZB1:RW_ZB1 + G * 128]
        psZ = ps_sm.tile([128, 4, 128], F32, tag="sm")
        for k in range(G):
            mm(psZ[:, k, :], ones1, zb1[:, k * 128:(k + 1) * 128],
               start=True, stop=False)
            mm(psZ[:, k, :], hnT[:, w0 + k, :], W1b, start=False, stop=True)
        z = work.tile([128, G, 128], BF16, tag="z")
        nc.scalar.copy(z[:], psZ[:, 0:G, :])
        nc.sync.dma_start(
            zin[w0 * 128:(w0 + G) * 128, :].rearrange("(g p) c -> p g c", p=128),
            z[:])

    def allgather(zin, zout):
        if sim1 or NO_CC:
            for c in range(CORES):
                nc.sync.dma_start(zout[c * NPC:(c + 1) * NPC, :], zin[:])
            return
        nc.gpsimd.collective_compute(
            "AllGather", ALU.bypass,
            replica_groups=[list(range(CORES))],
            ins=[zin.opt()], outs=[zout.opt()],
        )

    # ---- encoders ----
    encn_b1, encn_b2 = encC[:, 0:1], encC[:, 1:2]
    encn_g, encn_beta = encC[:, 2:3], encC[:, 3:4]
    ence_b1, ence_b2 = encC[:, 4:5], encC[:, 5:6]
    ence_g, ence_beta = encC[:, 6:7], encC[:, 7:8]
    dec_b1, dec_b2 = encC[:, 8:9], encC[:, 9:10]

    wts = {0: weight_tiles(0)} if n_layers > 0 else {}
    if n_layers > 0:
        zin, zout = z_alloc()
    # node encoder: feature-major in, node-major ps3, LN, transpose -> hnT
    for w0, G in NGRP:
        ps3 = mlp3([(encnW1[:], lambda a, b: nfT[:, w0 + a:w0 + b, :])],
                   encn_b1, cs(encW, 0), encn_b2, cs(encW, 1),
                   encRw[:, ERW_B3N:ERW_B3N + G * 128], G)
        A, B = ln_ab(ps3, G)
        xng = norm(ps3, A, B, G)
        psT = transpose_g(xng, G)
        nc.scalar.activation(hnT[:, w0:w0 + G, :], psT[:], ACT_F.Identity,
                             bias=encn_beta, scale=encn_g)
        if n_layers > 0:
            z_group(w0, G, zin, wts[0])
    # edge encoder
    for t0, G in EGRP:
        ps3 = mlp3([(enceW1[:], lambda a, b: efT[:, t0 + a:t0 + b, :])],
                   ence_b1, cs(encW, 2), ence_b2, cs(encW, 3),
                   encRw[:, ERW_B3E:ERW_B3E + G * 128], G)
        A, B = ln_ab(ps3, G)
        xng = norm(ps3, A, B, G)
        psT = transpose_g(xng, G)
        nc.scalar.activation(he_fm[:, t0:t0 + G, :], psT[:], ACT_F.Identity,
                             bias=ence_beta, scale=ence_g)

    if n_layers > 0:
        allgather(zin, zout)

    # ---- message-passing layers ----
    for l in range(n_layers):
        wt = wts[l]
        Wp, Rw, Cc, R1 = wt["Wp"], wt["Rw"], wt["Cc"], wt["R1"]
        W1a, W1c = cs(Wp, 0), cs(Wp, 1)
        W2e, W3e = cs(Wp, 3), cs(Wp, 4)
        W1n0, W1n1 = cs(Wp, 5), cs(Wp, 6)
        W2n, W3n = cs(Wp, 7), cs(Wp, 8)
        b2e_col, b1n_col, b2n_col = Cc[:, 0:1], Cc[:, 1:2], Cc[:, 2:3]
        ge_col, gn_col = Cc[:, 3:4], Cc[:, 4:5]
        ey1b_col = Cc[:, 5:6]
        b3e_row = Rw[:, RW_B3E:RW_B3E + 768]
        b3n_row = Rw[:, RW_B3N:RW_B3N + 512]
        bW1_row = R1[:, 0:128]
        last = l == n_layers - 1
        if not last:
            wts[l + 1] = weight_tiles(l + 1)

        # ---- edge phase ----
        for w in range(WPC):
            t0 = w * TPW
            # sender gather, feature-major [128, 1, CAP]
            xsT = xpool.tile([128, 1, CAP], BF16, tag="xsT")
            if NO_GATHER:
                nc.sync.dma_start(
                    xsT[:, 0, :].rearrange("p (t q) -> p t q", t=TPW),
                    zout[0:CAP, :].rearrange("(t q) p -> p t q", q=128))
            else:
                nc.gpsimd.dma_gather(
                    xsT[:], zout[:],
                    sidx[:, w * (CAP // 16):(w + 1) * (CAP // 16)],
                    CAP, CAP, 128, transpose=True,
                )
            # receiver pre-transform Rn = hn @ W1c (node-major)
            psRn = ps_sm.tile([128, 4, 128], F32, tag="sm")
            mm(psRn[:, 0, :], hnT[:, w, :], W1c)
            rn = work.tile([128, 128], BF16, tag="rn")
            nc.scalar.copy(rn[:], psRn[:, 0, :])
            ps3 = mlp3(
                [(W1a, lambda a, b: he_fm[:, t0 + a:t0 + b, :]),
                 (rn[:], lambda a, b: OT_sb[:, t0 + a:t0 + b, :]),
                 (ident[:], lambda a, b: xsT[:, 0, a * 128:b * 128])],
                ey1b_col, W2e, b2e_col, W3e, b3e_row, TPW)
            A, B = ln_ab(ps3, TPW)
            xng = norm(ps3, A, B, TPW)
            # scatter-add into psA (feature-major out), evac with g scale
            psA = ps_sm.tile([128, 4, 128], F32, tag="sm")
            for i in range(TPW):
                mm(psA[:, 0, :], xng[:, i, :], O_sb[:, t0 + i, :],
                   start=(i == 0), stop=(i == TPW - 1))
            nc.vector.tensor_scalar(aggT[:, w, :], psA[:, 0, :], ge_col,
                                    None, ALU.mult)
            # he += (xng + beta/g)^T * g  (fp16 state)
            psT = transpose_g(xng, TPW)
            nc.vector.scalar_tensor_tensor(
                he_fm[:, t0:t0 + TPW, :], psT[:], ge_col,
                he_fm[:, t0:t0 + TPW, :], ALU.mult, ALU.add)

        # ---- node phase ----
        if not last:
            zin, zout_next = z_alloc()
        for w0, G in NGRP:
            ps3 = mlp3(
                [(W1n0, lambda a, b: hnT[:, w0 + a:w0 + b, :]),
                 (W1n1, lambda a, b: aggT[:, w0 + a:w0 + b, :]),
                 (bW1_row, lambda a, b: deg[:, (w0 + a) * 128:(w0 + b) * 128])],
                b1n_col, W2n, b2n_col, W3n, b3n_row[:, 0:G * 128], G)
            A, B = ln_ab(ps3, G)
            xng = norm(ps3, A, B, G)
            psT = transpose_g(xng, G)
            nc.vector.scalar_tensor_tensor(
                hnT[:, w0:w0 + G, :], psT[:], gn_col,
                hnT[:, w0:w0 + G, :], ALU.mult, ALU.add)
            if not last:
                z_group(w0, G, zin, wts[l + 1])
        if not last:
            allgather(zin, zout_next)
            zout = zout_next

    # ---- decoder (fp16) ----
    for w0, G in NGRP:
        ps1 = ps_mlp.tile([128, G, 128], F32, tag="mlp")
        mm(ps1[:], cs(encW, 4), hnT[:, w0:w0 + G, :])
        d1 = work.tile([128, G, 128], BF16, tag="y1")
        nc.scalar.activation(d1[:], ps1[:], ACT_F.Relu, bias=dec_b1)
        ps2 = ps_mlp.tile([128, G, 128], F32, tag="mlp")
        mm(ps2[:], cs(encW, 5), d1[:])
        d2 = work.tile([128, G, 128], BF16, tag="y2")
        nc.scalar.activation(d2[:], ps2[:], ACT_F.Relu, bias=dec_b2)
        ps3 = ps_mlp.tile([128, G, 128], F32, tag="mlp")
        for i in range(G):
            mm(ps3[:, i, 0:3], ones1, encRw[:, ERW_B3D + i * 3:ERW_B3D + (i + 1) * 3],
               start=True, stop=False)
            mm(ps3[:, i, 0:3], d2[:, i, :], decW3[:], start=False, stop=True)
        d3 = work.tile([128, G, 3], F32, tag="d3")
        nc.scalar.copy(d3[:], ps3[:, :, 0:3])
        nc.sync.dma_start(
            out_d[w0 * 128:(w0 + G) * 128, :].rearrange("(g p) c -> p g c", p=128),
            d3[:])

    st.close()


# ----------------------------------------------------------------------------
# Entry point
# ----------------------------------------------------------------------------

_NC_CACHE = {}


def kernel(**inputs):
    perm_pos, send_new, recv_new, edge_slots = pack_graph(inputs["edge_index"])
    in_maps = build_inputs(inputs, perm_pos, send_new, recv_new, edge_slots)

    if "nc" not in _NC_CACHE:
        _NC_CACHE["nc"] = build_program(NL)
    nc = _NC_CACHE["nc"]

    res = run_bass_kernel_spmd(nc, in_maps, list(range(CORES)))
    _NC_CACHE["last_results"] = res
    out_pad = np.concatenate([r["out"] for r in res.results], axis=0)
    return np.ascontiguousarray(out_pad[perm_pos]).astype(np.float32)


if __name__ == "__main__":
    sys.path.insert(0, "/root/problem")
    import reference
    inp = {k: np.asarray(v) for k, v in reference.setup_inputs().items()}
    got = kernel(**inp)
    exp = np.asarray(reference.reference(**inp))
    rel = np.abs(got - exp).max() / (np.abs(exp).max() + 1e-12)
    print("rel(absmax) =", rel)


# revision 18
# speedup vs baseline: 1.0611x; 1.0611x over previous
"""MeshGraphNet on 8 Trainium2 NeuronCores (Bass/Tile, SPMD).

Strategy (see spec sharding_hint): edges partitioned across cores by
*receiver* node; node space padded 10000 -> 10240 and bin-packed into
8 cores x 10 windows x 128 nodes so each window owns <= 768 incident edges
(6 edge-tiles of 128). All matmul data is fp16 (PE: 1 cycle/row, 8x finer
mantissa than bf16) and flows feature-major: MLP weights are the
stationary lhsT and the activations are wide moving rhs. Receiver gather /
scatter-add are one-hot matmuls fused into the PSUM accumulation of the
consuming/producing MLP stage. The sender gather runs on the
W1b-transformed node embedding Z = hn @ pe_W1[128:256] + b1, AllGather'ed
to DRAM once per layer (fp16) and row-gathered by sender index via SWDGE
dma_gather in transpose mode.

Engine diet vs the v1 kernel: each edge window is ONE G=6 MLP group;
all rank-1 biases (b3, zb1, beta/g) are folded into PSUM accumulations
as K=1 matmuls; LayerNorm stats come from tensor_reduce (sum) + squared
tensor_tensor + tensor_reduce instead of per-tile bn_stats/bn_aggr; the
normalize is y*A + B with per-edge A/B and the LN gain g is folded into
the consumers (aggT evacuation scale / residual STT scale), so the DVE
does 5 full passes per window instead of ~10. Residual state is fp16.
"""

import os
import sys

import numpy as np

for _p in ("/opt/trn_rl_repo", "/root/.axon_site/_ro/trn_rl_repo"):
    if os.path.isdir(_p) and _p not in sys.path:
        sys.path.insert(0, _p)

import concourse.bass as bass
import concourse.bacc as bacc
import concourse.mybir as mybir
import concourse.tile as tile
from concourse.bass_utils import run_bass_kernel_spmd

F32 = mybir.dt.float32
BF16 = mybir.dt.float16  # 16-bit compute dtype (fp16: 1 cyc/row on PE)
I16 = mybir.dt.int16
ALU = mybir.AluOpType
ACT_F = mybir.ActivationFunctionType

N, E, D = 10000, 60000, 128
NF, EF, NL = 12, 3, 15
CORES = 8
WPC = 10                       # windows per core
NPC = WPC * 128                # 1280 nodes per core
NPAD = CORES * NPC             # 10240
TPW = 6                        # edge tiles per window
CAP = TPW * 128                # 768 edges per window max
T = WPC * TPW                  # 60 edge tiles per core
P = T * 128                    # 7680 edge slots per core
EPS = 1e-5

NGRP = [(0, 4), (4, 3), (7, 3)]             # node window groups
EGRP = [(t0, 6) for t0 in range(0, T, 6)]   # edge-encoder tile groups

# Rw row-constant layout (fp16 [1, 1792] per layer)
RW_B3E = 0          # b3e tiled 6x  [768]
RW_B3N = 768        # b3n tiled 4x  [512]
RW_ZB1 = 1280       # adjusted pe_b1 tiled 4x [512]
RW_LEN = 1792

# encRw row-constant layout (fp16 [1, 1552])
ERW_B3N = 0         # enc_n_b3 tiled 6x [768]
ERW_B3E = 768       # enc_e_b3 tiled 6x [768]
ERW_B3D = 1536      # dec_b3 tiled 4x [12] (+pad)
ERW_LEN = 1552


# ----------------------------------------------------------------------------
# Host-side graph packing
# ----------------------------------------------------------------------------

def pack_graph(edge_index):
    send0 = np.asarray(edge_index[0], np.int64)
    recv0 = np.asarray(edge_index[1], np.int64)
    deg = np.bincount(recv0, minlength=N)

    order = np.argsort(-deg, kind="stable")
    nwin = CORES * WPC
    win_fill = np.zeros(nwin, dtype=np.int64)    # node count per window
    win_load = np.zeros(nwin, dtype=np.int64)    # edge count per window
    perm_pos = np.full(N, -1, dtype=np.int64)
    for nid in order:
        d = deg[nid]
        cand = np.nonzero((win_fill < 128) & (win_load + d <= CAP))[0]
        assert len(cand), "window packing failed (need TPW bump)"
        w = cand[np.argmin(win_load[cand])]
        perm_pos[nid] = w * 128 + win_fill[w]
        win_fill[w] += 1
        win_load[w] += d
    assert (perm_pos >= 0).all()

    send_new = perm_pos[send0]
    recv_new = perm_pos[recv0]

    ewin = recv_new // 128
    edge_slots = np.full((CORES, P), -1, dtype=np.int64)
    for c in range(CORES):
        for wl in range(WPC):
            w = c * WPC + wl
            eids = np.nonzero(ewin == w)[0]
            base = wl * CAP
            edge_slots[c, base : base + len(eids)] = eids
    return perm_pos, send_new, recv_new, edge_slots


def _col(v):
    """[K] -> [K, 1] fp32 column (per-partition bias/scale)."""
    return np.asarray(v, np.float32).reshape(-1, 1).copy()


def build_inputs(inp, perm_pos, send_new, recv_new, edge_slots):
    """Build in_maps (one dict per core) for the device program."""
    g32 = lambda k: np.ascontiguousarray(np.asarray(inp[k], np.float32))
    bf = lambda a: np.asarray(a, np.float32).astype(np.float16)

    nf_pad = np.zeros((NPAD, NF), np.float32)
    nf_pad[perm_pos] = g32("node_features")
    ef = g32("edge_features")

    peW1, peW2, peW3 = g32("pe_W1"), g32("pe_W2"), g32("pe_W3")
    pnW1, pnW2, pnW3 = g32("pn_W1"), g32("pn_W2"), g32("pn_W3")

    # per-layer stationary/moving weights, fp16, [NL, 128, 9*128]
    Wp = np.stack([
        np.concatenate([
            peW1[l, 0:128], peW1[l, 256:384], peW1[l, 128:256],
            peW2[l], peW3[l],
            pnW1[l, 0:128], pnW1[l, 128:256], pnW2[l], pnW3[l],
        ], axis=1)
        for l in range(NL)
    ])  # order: W1a W1c W1b W2e W3e W1n0 W1n1 W2n W3n

    def tl(v, n):
        return np.tile(np.asarray(v, np.float32).reshape(1, -1), (1, n))

    # Device state is beta-free: he'_l = he_l - C_l, hn'_l = hn_l - D_l
    # with C_l = sum_{k<l} pe_beta_k, D_l = sum_{k<l} pn_beta_k. The
    # constant parts are folded into downstream bias columns here.
    betae = np.asarray(inp["pe_beta"], np.float64)
    betan = np.asarray(inp["pn_beta"], np.float64)
    Cl = np.concatenate([np.zeros((1, D)), np.cumsum(betae, 0)])[:NL]
    Dl = np.concatenate([np.zeros((1, D)), np.cumsum(betan, 0)])[:NL]
    D_NL = betan.sum(0)

    # per-layer fp16 rows [1, RW_LEN]
    Rw = np.stack([
        np.concatenate([
            tl(inp["pe_b3"][l], 6), tl(inp["pn_b3"][l], 4),
            tl(np.asarray(inp["pe_b1"][l], np.float64)
               + Dl[l] @ np.asarray(peW1[l, 128:256], np.float64), 4),
        ], axis=1)
        for l in range(NL)
    ])  # [NL, 1, RW_LEN]

    # per-layer fp32 columns [128, 6]: b2e, b1n', b2n, ge, gn, ey1b
    Cc = np.stack([
        np.concatenate([
            _col(inp["pe_b2"][l]),
            _col(np.asarray(inp["pn_b1"][l], np.float64)
                 + Dl[l] @ np.asarray(pnW1[l, 0:128], np.float64)),
            _col(inp["pn_b2"][l]),
            _col(inp["pe_g"][l]), _col(inp["pn_g"][l]),
            _col(Cl[l] @ np.asarray(peW1[l, 0:128], np.float64)
                 + Dl[l] @ np.asarray(peW1[l, 256:384], np.float64)),
        ], axis=1)
        for l in range(NL)
    ])
    # per-layer single-partition row [1, 128]: betaW1 (deg compensation)
    betaW1 = np.stack([
        np.asarray(inp["pe_beta"][l], np.float64) @ np.asarray(pnW1[l, 128:256], np.float64)
        for l in range(NL)
    ]).astype(np.float32)
    R1 = betaW1.reshape(NL, 1, 128)

    encRw = np.concatenate([
        tl(inp["enc_n_b3"], 6), tl(inp["enc_e_b3"], 6),
        tl(inp["dec_b3"], 4), np.zeros((1, 4), np.float32),
    ], axis=1)  # [1, ERW_LEN]

    shared = {
        "Wp": bf(Wp), "Rw": bf(Rw), "Cc": Cc, "R1": bf(R1),
        # encoders / decoder (fp16, like the layer loop)
        "encnW1": bf(g32("enc_n_W1")),
        "enceW1": bf(g32("enc_e_W1")),
        "encW": bf(np.concatenate([
            g32("enc_n_W2"), g32("enc_n_W3"),
            g32("enc_e_W2"), g32("enc_e_W3"),
            g32("dec_W1"), g32("dec_W2"),
        ], axis=1)),  # [128, 6*128]
        "decW3": bf(g32("dec_W3")),
        # [128, 10] fp32: encn_b1 b2 g beta | ence_b1 b2 g beta | dec_b1' b2
        "encC": np.concatenate([
            _col(inp["enc_n_b1"]), _col(inp["enc_n_b2"]),
            _col(inp["enc_n_g"]), _col(inp["enc_n_beta"]),
            _col(inp["enc_e_b1"]), _col(inp["enc_e_b2"]),
            _col(inp["enc_e_g"]), _col(inp["enc_e_beta"]),
            _col(np.asarray(inp["dec_b1"], np.float64)
                 + D_NL @ np.asarray(g32("dec_W1"), np.float64)),
            _col(inp["dec_b2"]),
        ], axis=1),
        "encRw": bf(encRw),
        "ones": bf(np.ones((1, CAP), np.float32)),
        "ident": bf(np.eye(128, dtype=np.float32)),
    }

    in_maps = []
    for c in range(CORES):
        sl = edge_slots[c]
        v = sl >= 0
        send_c = np.zeros(P, np.int64)
        send_c[v] = send_new[sl[v]]
        recv_c = np.zeros(P, np.int64)
        recv_c[v] = recv_new[sl[v]]

        efT = np.zeros((EF, P), np.float32)
        efT[:, v] = ef[sl[v]].T

        slots = np.arange(P)
        t, p = slots // 128, slots % 128
        j = recv_c - (c * NPC + (t // TPW) * 128)
        O = np.zeros((128, P), np.float32)
        OT = np.zeros((128, P), np.float32)
        O[p[v], t[v] * 128 + j[v]] = 1.0
        OT[j[v], t[v] * 128 + p[v]] = 1.0

        wrapped = send_c.astype(np.int16).reshape(P // 16, 16).T  # [16, P//16]
        sidx = np.tile(wrapped, (8, 1))                           # [128, P//16]

        # per-local-node incident edge count (receiver degree), [1, NPC]
        deg_c = np.zeros(NPC, np.float32)
        np.add.at(deg_c, recv_c[v] - c * NPC, 1.0)

        m = dict(shared)
        m.update({
            "nfT": bf(np.ascontiguousarray(
                nf_pad[c * NPC:(c + 1) * NPC].T).reshape(NF, WPC, 128)),
            "efT": bf(efT.reshape(EF, T, 128)),
            "O": bf(O.reshape(128, T, 128)),
            "OT": bf(OT.reshape(128, T, 128)),
            "sidx": sidx,
            "deg": bf(deg_c.reshape(1, NPC)),
        })
        in_maps.append(m)
    return in_maps


# ----------------------------------------------------------------------------
# Device program
# ----------------------------------------------------------------------------

def build_program(n_layers=NL, sim1=False):
    nc = bacc.Bacc("TRN2", target_bir_lowering=False, debug=False,
                   num_devices=1 if sim1 else CORES)

    dram = {}

    def din(name, shape, dt=BF16):
        dram[name] = nc.dram_tensor(name, list(shape), dt, kind="ExternalInput")
        return dram[name]

    din("nfT", [NF, WPC, 128])
    din("efT", [EF, T, 128])
    din("O", [128, T, 128])
    din("OT", [128, T, 128])
    din("sidx", [128, P // 16], I16)
    din("deg", [1, NPC])
    din("Wp", [NL, 128, 9 * 128])
    din("Rw", [NL, 1, RW_LEN])
    din("Cc", [NL, 128, 6], F32)
    din("R1", [NL, 1, 128])
    din("encnW1", [NF, 128])
    din("enceW1", [EF, 128])
    din("encW", [128, 6 * 128])
    din("decW3", [128, 3])
    din("encC", [128, 10], F32)
    din("encRw", [1, ERW_LEN])
    din("ones", [1, CAP])
    din("ident", [128, 128])
    out_d = nc.dram_tensor("out", [NPC, 3], F32, kind="ExternalOutput")

    with tile.TileContext(nc) as tc:
        _build_tile_program(nc, tc, dram, out_d, n_layers, sim1)
    nc.compile()
    return nc


def _build_tile_program(nc, tc, dram, out_d, n_layers, sim1=False):
    from contextlib import ExitStack

    st = ExitStack()
    const = st.enter_context(tc.tile_pool(name="const", bufs=1))
    wpool = st.enter_context(tc.tile_pool(name="wpool", bufs=2))
    work = st.enter_context(tc.tile_pool(name="work", bufs=4))
    xpool = st.enter_context(tc.tile_pool(name="xpool", bufs=6))
    ps_mlp = st.enter_context(tc.tile_pool(name="ps_mlp", bufs=2, space="PSUM"))
    ps_tp = st.enter_context(tc.tile_pool(name="ps_tp", bufs=2, space="PSUM"))
    ps_sm = st.enter_context(tc.tile_pool(name="ps_sm", bufs=2, space="PSUM"))
    dpool = st.enter_context(tc.tile_pool(name="dram", bufs=2, space="DRAM"))

    NO_CC = bool(int(os.environ.get("K_NO_CC", "0")))
    NO_GATHER = bool(int(os.environ.get("K_NO_GATHER", "0")))
    zspace = "Local" if (sim1 or NO_CC) else "Shared"

    def mm(out, lhsT, rhs, start=True, stop=True):
        nc.tensor.matmul(out, lhsT, rhs, start=start, stop=stop)

    def cs(ap2, c):   # chunk slice: [..., K, 128*nc] -> cols of chunk c
        return ap2[:, c * 128:(c + 1) * 128]

    # ---- resident SBUF state ----
    ident = const.tile([128, 128], BF16)
    nc.sync.dma_start(ident[:], dram["ident"][:])
    ones = const.tile([1, CAP], BF16)
    nc.sync.dma_start(ones[:], dram["ones"][:])
    deg = const.tile([1, NPC], BF16)
    nc.sync.dma_start(deg[:], dram["deg"][:])
    O_sb = const.tile([128, T, 128], BF16)
    nc.sync.dma_start(O_sb[:], dram["O"][:])
    OT_sb = const.tile([128, T, 128], BF16)
    nc.sync.dma_start(OT_sb[:], dram["OT"][:])
    sidx = const.tile([128, P // 16], I16)
    nc.sync.dma_start(sidx[:], dram["sidx"][:])
    eps_col = const.tile([128, 1], F32)
    nc.vector.memset(eps_col[:], EPS)
    zeros_col = const.tile([128, 1], BF16)
    nc.vector.memset(zeros_col[:], 0.0)
    he_fm = const.tile([128, T, 128], BF16)    # edge state, feature-major
    hnT = const.tile([128, WPC, 128], BF16)    # node state, feature-major
    aggT = const.tile([128, WPC, 128], BF16)   # scatter result, feature-major

    # ---- encoder/decoder weights (fp16) ----
    encnW1 = const.tile([NF, 128], BF16)
    nc.sync.dma_start(encnW1[:], dram["encnW1"][:])
    enceW1 = const.tile([EF, 128], BF16)
    nc.sync.dma_start(enceW1[:], dram["enceW1"][:])
    encW = const.tile([128, 6 * 128], BF16)
    nc.sync.dma_start(encW[:], dram["encW"][:])
    decW3 = const.tile([128, 3], BF16)
    nc.sync.dma_start(decW3[:], dram["decW3"][:])
    encC = const.tile([128, 10], F32)
    nc.sync.dma_start(encC[:], dram["encC"][:])
    encRw = const.tile([1, ERW_LEN], BF16)
    nc.sync.dma_start(encRw[:], dram["encRw"][:])
    nfT = const.tile([NF, WPC, 128], BF16)
    nc.sync.dma_start(nfT[:], dram["nfT"][:])
    efT = const.tile([EF, T, 128], BF16)
    nc.sync.dma_start(efT[:], dram["efT"][:])
    ones1 = ones[:, 0:128]

    def weight_tiles(l):
        Wp = wpool.tile([128, 9 * 128], BF16, tag="Wp")
        nc.sync.dma_start(Wp[:], dram["Wp"][l])
        Rw = wpool.tile([1, RW_LEN], BF16, tag="Rw")
        nc.sync.dma_start(Rw[:], dram["Rw"][l])
        Cc = wpool.tile([128, 6], F32, tag="Cc")
        nc.sync.dma_start(Cc[:], dram["Cc"][l])
        R1 = wpool.tile([1, 128], BF16, tag="R1")
        nc.sync.dma_start(R1[:], dram["R1"][l])
        return {"Wp": Wp, "Rw": Rw, "Cc": Cc, "R1": R1}

    def mlp3(parts, b1_col, W2, b2_col, W3, b3_row, G):
        """3-stage MLP on G tiles; stage-1 inputs are feature-major
        (lhsT, slicer) pairs accumulated into one PSUM, where
        slicer(a, b) gives the moving operand for tile range [a, b).
        Matmuls are chunked to <=512 fp32 out cols (one PSUM bank).
        Output ps3 is edge/node-major [128, G, 128] fp32 PSUM with b3
        already added (b3_row is [1, G*128], a K=1 rank-1 term)."""
        chunks = [(0, min(G, 4))] + ([(4, G)] if G > 4 else [])
        ps1 = ps_mlp.tile([128, G, 128], F32, tag="mlp")
        nmm = len(parts)
        for a, b in chunks:
            for i, (lt, sl) in enumerate(parts):
                mm(ps1[:, a:b, :], lt, sl(a, b),
                   start=(i == 0), stop=(i == nmm - 1))
        y1 = work.tile([128, G, 128], BF16, tag="y1")
        b1s = 0.0 if b1_col is None else b1_col
        nc.vector.scalar_tensor_tensor(
            y1[:], ps1[:], b1s, zeros_col[:, None, :].to_broadcast([128, G, 128]),
            ALU.add, ALU.max)
        ps2 = ps_mlp.tile([128, G, 128], F32, tag="mlp")
        for a, b in chunks:
            mm(ps2[:, a:b, :], W2, y1[:, a:b, :])
        y2 = work.tile([128, G, 128], BF16, tag="y2")
        nc.scalar.activation(y2[:], ps2[:], ACT_F.Relu, bias=b2_col)
        ps3 = ps_mlp.tile([128, G, 128], F32, tag="mlp")
        for a, b in chunks:
            mm(ps3[:, a:b, :], ones1, b3_row[:, a * 128:b * 128],
               start=True, stop=False)
        for i in range(G):
            mm(ps3[:, i, :], y2[:, i, :], W3, start=False, stop=True)
        y3 = work.tile([128, G, 128], BF16, tag="y3")
        nc.scalar.copy(y3[:], ps3[:])
        return y3

    def ln_ab(y3, G):
        """LN coefficients per row of [128, G, 128]: A = rstd,
        B = -mean*rstd, both [128, G, 1] fp32."""
        s1 = work.tile([128, G, 1], F32, tag="s1")
        nc.vector.tensor_reduce(s1[:], y3[:], mybir.AxisListType.X, ALU.add)
        sq = work.tile([128, G, 128], BF16, tag="sq")
        nc.vector.tensor_tensor(sq[:], y3[:], y3[:], ALU.mult)
        s2 = work.tile([128, G, 1], F32, tag="s2")
        nc.vector.tensor_reduce(s2[:], sq[:], mybir.AxisListType.X, ALU.add)
        # var*128^2 = 128*sumsq - sum^2
        t1 = work.tile([128, G, 1], F32, tag="t1")
        nc.vector.tensor_tensor(t1[:], s1[:], s1[:], ALU.mult)
        vs = work.tile([128, G, 1], F32, tag="vs")
        nc.vector.scalar_tensor_tensor(vs[:], s2[:], 128.0, t1[:],
                                       ALU.mult, ALU.subtract)
        sd = work.tile([128, G, 1], F32, tag="sd")
        nc.scalar.activation(sd[:], vs[:], ACT_F.Sqrt, bias=eps_col[:],
                             scale=1.0 / 16384.0)
        A = work.tile([128, G, 1], F32, tag="A")
        nc.vector.reciprocal(A[:], sd[:])
        B = work.tile([128, G, 1], F32, tag="B")
        nc.vector.scalar_tensor_tensor(B[:], s1[:], -1.0 / 128.0, A[:],
                                       ALU.mult, ALU.mult)
        return A, B

    def norm(y3, A, B, G):
        """(y3 - mean) * rstd -> fp16 [128, G, 128] (no LN gain g)."""
        xng = work.tile([128, G, 128], BF16, tag="xng")
        nc.vector.tensor_tensor(
            xng[:], y3[:], A[:].to_broadcast([128, G, 128]), ALU.mult)
        nc.vector.tensor_tensor(
            xng[:], xng[:], B[:].to_broadcast([128, G, 128]), ALU.add)
        return xng

    def transpose_g(xng, G):
        """psT = xng^T per tile, [128, G, 128] fp16 PSUM (feature-major)."""
        psT = ps_tp.tile([128, G, 128], BF16, tag="tp")
        for i in range(G):
            nc.tensor.transpose(psT[:, i, :], xng[:, i, :], ident[:])
        return psT

    def z_alloc():
        zin = dpool.tile([NPC, 128], BF16, tag="zin")
        zout = dpool.tile([NPAD, 128], BF16, tag="zout", addr_space=zspace)
        return zin, zout

    def z_group(w0, G, zin, wt):
        """Z = hn @ W1b + b1e for one node group -> zin rows."""
        W1b = cs(wt["Wp"], 2)
        zb1 = wt["Rw"][:, RW_ZB1:RW_ZB1 + G * 128]
        psZ = ps_sm.tile([128, 4, 128], F32, tag="sm")
        mm(psZ[:, 0:G, :], ones1, zb1[:], start=True, stop=False)
        for k in range(G):
            mm(psZ[:, k, :], hnT[:, w0 + k, :], W1b, start=False, stop=True)
        z = work.tile([128, G, 128], BF16, tag="z")
        nc.scalar.copy(z[:], psZ[:, 0:G, :])
        nc.sync.dma_start(
            zin[w0 * 128:(w0 + G) * 128, :].rearrange("(g p) c -> p g c", p=128),
            z[:])

    def allgather(zin, zout):
        if sim1 or NO_CC:
            for c in range(CORES):
                nc.sync.dma_start(zout[c * NPC:(c + 1) * NPC, :], zin[:])
            return
        nc.gpsimd.collective_compute(
            "AllGather", ALU.bypass,
            replica_groups=[list(range(CORES))],
            ins=[zin.opt()], outs=[zout.opt()],
        )

    # ---- encoders ----
    encn_b1, encn_b2 = encC[:, 0:1], encC[:, 1:2]
    encn_g, encn_beta = encC[:, 2:3], encC[:, 3:4]
    ence_b1, ence_b2 = encC[:, 4:5], encC[:, 5:6]
    ence_g, ence_beta = encC[:, 6:7], encC[:, 7:8]
    dec_b1, dec_b2 = encC[:, 8:9], encC[:, 9:10]

    wts = {0: weight_tiles(0)} if n_layers > 0 else {}
    if n_layers > 0:
        zin, zout = z_alloc()
    # node encoder: feature-major in, node-major ps3, LN, transpose -> hnT
    for w0, G in NGRP:
        y3 = mlp3([(encnW1[:], lambda a, b: nfT[:, w0 + a:w0 + b, :])],
                  encn_b1, cs(encW, 0), encn_b2, cs(encW, 1),
                  encRw[:, ERW_B3N:ERW_B3N + G * 128], G)
        A, B = ln_ab(y3, G)
        xng = norm(y3, A, B, G)
        psT = transpose_g(xng, G)
        nc.scalar.activation(hnT[:, w0:w0 + G, :], psT[:], ACT_F.Identity,
                             bias=encn_beta, scale=encn_g)
        if n_layers > 0:
            z_group(w0, G, zin, wts[0])
    # edge encoder
    for t0, G in EGRP:
        y3 = mlp3([(enceW1[:], lambda a, b: efT[:, t0 + a:t0 + b, :])],
                  ence_b1, cs(encW, 2), ence_b2, cs(encW, 3),
                  encRw[:, ERW_B3E:ERW_B3E + G * 128], G)
        A, B = ln_ab(y3, G)
        xng = norm(y3, A, B, G)
        psT = transpose_g(xng, G)
        nc.scalar.activation(he_fm[:, t0:t0 + G, :], psT[:], ACT_F.Identity,
                             bias=ence_beta, scale=ence_g)

    if n_layers > 0:
        allgather(zin, zout)

    # ---- message-passing layers ----
    for l in range(n_layers):
        wt = wts[l]
        Wp, Rw, Cc, R1 = wt["Wp"], wt["Rw"], wt["Cc"], wt["R1"]
        W1a, W1c = cs(Wp, 0), cs(Wp, 1)
        W2e, W3e = cs(Wp, 3), cs(Wp, 4)
        W1n0, W1n1 = cs(Wp, 5), cs(Wp, 6)
        W2n, W3n = cs(Wp, 7), cs(Wp, 8)
        b2e_col, b1n_col, b2n_col = Cc[:, 0:1], Cc[:, 1:2], Cc[:, 2:3]
        ge_col, gn_col = Cc[:, 3:4], Cc[:, 4:5]
        ey1b_col = Cc[:, 5:6]
        b3e_row = Rw[:, RW_B3E:RW_B3E + 768]
        b3n_row = Rw[:, RW_B3N:RW_B3N + 512]
        bW1_row = R1[:, 0:128]
        last = l == n_layers - 1
        if not last:
            wts[l + 1] = weight_tiles(l + 1)

        # ---- edge phase ----
        for w in range(WPC):
            t0 = w * TPW
            # sender gather, feature-major [128, 1, CAP]
            xsT = xpool.tile([128, 1, CAP], BF16, tag="xsT")
            if NO_GATHER:
                nc.sync.dma_start(
                    xsT[:, 0, :].rearrange("p (t q) -> p t q", t=TPW),
                    zout[0:CAP, :].rearrange("(t q) p -> p t q", q=128))
            else:
                nc.gpsimd.dma_gather(
                    xsT[:], zout[:],
                    sidx[:, w * (CAP // 16):(w + 1) * (CAP // 16)],
                    CAP, CAP, 128, transpose=True,
                )
            # receiver pre-transform Rn = hn @ W1c (node-major)
            psRn = ps_sm.tile([128, 4, 128], F32, tag="sm")
            mm(psRn[:, 0, :], hnT[:, w, :], W1c)
            rn = work.tile([128, 128], BF16, tag="rn")
            nc.scalar.copy(rn[:], psRn[:, 0, :])
            y3 = mlp3(
                [(W1a, lambda a, b: he_fm[:, t0 + a:t0 + b, :]),
                 (rn[:], lambda a, b: OT_sb[:, t0 + a:t0 + b, :]),
                 (ident[:], lambda a, b: xsT[:, 0, a * 128:b * 128])],
                ey1b_col, W2e, b2e_col, W3e, b3e_row, TPW)
            A, B = ln_ab(y3, TPW)
            xng = norm(y3, A, B, TPW)
            # scatter-add into psA (feature-major out), evac with g scale
            psA = ps_sm.tile([128, 4, 128], F32, tag="sm")
            for i in range(TPW):
                mm(psA[:, 0, :], xng[:, i, :], O_sb[:, t0 + i, :],
                   start=(i == 0), stop=(i == TPW - 1))
            nc.vector.tensor_scalar(aggT[:, w, :], psA[:, 0, :], ge_col,
                                    None, ALU.mult)
            # he += (xng + beta/g)^T * g  (fp16 state)
            psT = transpose_g(xng, TPW)
            nc.vector.scalar_tensor_tensor(
                he_fm[:, t0:t0 + TPW, :], psT[:], ge_col,
                he_fm[:, t0:t0 + TPW, :], ALU.mult, ALU.add)

        # ---- node phase ----
        if not last:
            zin, zout_next = z_alloc()
        for w0, G in NGRP:
            y3 = mlp3(
                [(W1n0, lambda a, b: hnT[:, w0 + a:w0 + b, :]),
                 (W1n1, lambda a, b: aggT[:, w0 + a:w0 + b, :]),
                 (bW1_row, lambda a, b: deg[:, (w0 + a) * 128:(w0 + b) * 128])],
                b1n_col, W2n, b2n_col, W3n, b3n_row[:, 0:G * 128], G)
            A, B = ln_ab(y3, G)
            xng = norm(y3, A, B, G)
            psT = transpose_g(xng, G)
            nc.vector.scalar_tensor_tensor(
                hnT[:, w0:w0 + G, :], psT[:], gn_col,
                hnT[:, w0:w0 + G, :], ALU.mult, ALU.add)
            if not last:
                z_group(w0, G, zin, wts[l + 1])
        if not last:
            allgather(zin, zout_next)
            zout = zout_next

    # ---- decoder (fp16) ----
    for w0, G in NGRP:
        ps1 = ps_mlp.tile([128, G, 128], F32, tag="mlp")
        mm(ps1[:], cs(encW, 4), hnT[:, w0:w0 + G, :])
        d1 = work.tile([128, G, 128], BF16, tag="y1")
        nc.scalar.activation(d1[:], ps1[:], ACT_F.Relu, bias=dec_b1)
        ps2 = ps_mlp.tile([128, G, 128], F32, tag="mlp")
        mm(ps2[:], cs(encW, 5), d1[:])
        d2 = work.tile([128, G, 128], BF16, tag="y2")
        nc.scalar.activation(d2[:], ps2[:], ACT_F.Relu, bias=dec_b2)
        ps3 = ps_mlp.tile([128, G, 128], F32, tag="mlp")
        for i in range(G):
            mm(ps3[:, i, 0:3], ones1, encRw[:, ERW_B3D + i * 3:ERW_B3D + (i + 1) * 3],
               start=True, stop=False)
            mm(ps3[:, i, 0:3], d2[:, i, :], decW3[:], start=False, stop=True)
        d3 = work.tile([128, G, 3], F32, tag="d3")
        nc.scalar.copy(d3[:], ps3[:, :, 0:3])
        nc.sync.dma_start(
            out_d[w0 * 128:(w0 + G) * 128, :].rearrange("(g p) c -> p g c", p=128),
            d3[:])

    st.close()


# ----------------------------------------------------------------------------
# Entry point
# ----------------------------------------------------------------------------

_NC_CACHE = {}


def kernel(**inputs):
    perm_pos, send_new, recv_new, edge_slots = pack_graph(inputs["edge_index"])
    in_maps = build_inputs(inputs, perm_pos, send_new, recv_new, edge_slots)

    if "nc" not in _NC_CACHE:
        _NC_CACHE["nc"] = build_program(NL)
    nc = _NC_CACHE["nc"]

    res = run_bass_kernel_spmd(nc, in_maps, list(range(CORES)))
    _NC_CACHE["last_results"] = res
    out_pad = np.concatenate([r["out"] for r in res.results], axis=0)
    return np.ascontiguousarray(out_pad[perm_pos]).astype(np.float32)


if __name__ == "__main__":
    sys.path.insert(0, "/root/problem")
    import reference
    inp = {k: np.asarray(v) for k, v in reference.setup_inputs().items()}
    got = kernel(**inp)
    exp = np.asarray(reference.reference(**inp))
    rel = np.abs(got - exp).max() / (np.abs(exp).max() + 1e-12)
    print("rel(absmax) =", rel)
